# revision 3
# baseline (speedup 1.0000x reference)
"""Multi-head attention (B=2, S=2048, H=2048, NH=16, HD=128) on 8 trn2 cores.

Sharding: core i -> (batch b = i // 4, head-group g = i % 4, 4 heads each).
Each core computes q/k/v projections for its 4 heads, causal-masked
attention, and a partial output projection against its 512-row slice of
Wo.  The host sums the 4 partial outputs per batch.

Layout strategy (everything K-major so no on-chip transposes are needed):
  - host ships x^T (per batch) in bf16; projections compute q^T/k^T
    [d, t] via lhsT=W, rhs=x^T, and v [T, d] via lhsT=x^T, rhs=Wv.
  - scores^T [T, t] = (k^T).T @ q^T; exp on ACT (no max subtraction --
    scores are O(6) here, exp is safe in fp32); runtime mask applied
    multiplicatively AFTER exp (so softmax denominators stay exact).
  - denominator via ones-vector matmul (partition-dim reduce on PE);
    o^T [d, t] = v.T @ e accumulates in PSUM; normalized by broadcast
    reciprocal on the way out to SBUF.
  - final: out[t, m] = (o^T).T @ Wo_rows, accumulated over the 4 heads.

The mask is inspected on the host and the kernel is specialized per
128x512 block: skip (all False), full (all True), or partial (loads the
mask tile and multiplies).  For the causal mask this halves attention
FLOPs; for an all-ones mask it becomes a dense kernel automatically.
"""

import math

import numpy as np
import ml_dtypes

B, S, H, NH, HD = 2, 2048, 2048, 16, 128
N_CORES = 8
GROUPS = 4                # head-groups (cores per batch)
HPC = NH // GROUPS        # heads per core = 4
DPC = HPC * HD            # head dims per core = 512
TBLK = 512                # query-block width (matmul moving dim)
KBLK = 128                # key-block width (matmul contraction dim)
NT = S // TBLK            # 4 query blocks
NK = S // KBLK            # 16 key blocks
HKT = H // 128            # 16 contraction tiles over hidden dim

_BF16 = ml_dtypes.bfloat16

_kernel_cache = {}


def _build(pattern):
    """Compile the SPMD program for a given mask block pattern.

    pattern: tuple over query-block tau of tuples of (Tb, partial) pairs,
    ascending in Tb, listing key blocks that have any visible entry.
    """
    import concourse.bass as bass  # noqa: F401
    import concourse.tile as tile
    from concourse import bacc, mybir

    fp32 = mybir.dt.float32
    bf16 = mybir.dt.bfloat16
    Exp = mybir.ActivationFunctionType.Exp
    inv_sqrt_hd = 1.0 / math.sqrt(HD)

    max_partial = max((sum(1 for _, p in blocks if p) for blocks in pattern),
                      default=0)

    nc = bacc.Bacc("TRN2", target_bir_lowering=False, debug=False,
                   num_devices=N_CORES)
    xT = nc.dram_tensor("xT", [H, S], bf16, kind="ExternalInput")
    wq = nc.dram_tensor("wq", [H, DPC], bf16, kind="ExternalInput")
    wk = nc.dram_tensor("wk", [H, DPC], bf16, kind="ExternalInput")
    wv = nc.dram_tensor("wv", [H, DPC], bf16, kind="ExternalInput")
    wo = nc.dram_tensor("wo", [DPC, H], bf16, kind="ExternalInput")
    maskT = nc.dram_tensor("maskT", [S, S], bf16, kind="ExternalInput")
    out = nc.dram_tensor("out", [S, H], fp32, kind="ExternalOutput")

    with tile.TileContext(nc) as tc:
        with (
            tc.tile_pool(name="persist", bufs=1) as persist,
            tc.tile_pool(name="xt", bufs=2) as xt_pool,
            tc.tile_pool(name="masks", bufs=max(2 * max_partial, 1)) as mask_pool,
            tc.tile_pool(name="e", bufs=6) as e_pool,
            tc.tile_pool(name="outsb", bufs=4) as out_pool,
            tc.tile_pool(name="small", bufs=4) as small_pool,
            tc.tile_pool(name="ps_work", bufs=4, space="PSUM") as ps_work,
            tc.tile_pool(name="ps_acc", bufs=2, space="PSUM") as ps_acc,
            tc.tile_pool(name="ps_den", bufs=2, space="PSUM") as ps_den,
        ):
            # --- persistent SBUF tensors -------------------------------
            wq_sb = persist.tile([128, HKT, DPC], bf16, tag="wq")
            wk_sb = persist.tile([128, HKT, DPC], bf16, tag="wk")
            wv_sb = persist.tile([128, HKT, DPC], bf16, tag="wv")
            nc.sync.dma_start(wq_sb[:], wq.ap().rearrange("(k p) d -> p k d", p=128))
            nc.sync.dma_start(wk_sb[:], wk.ap().rearrange("(k p) d -> p k d", p=128))
            nc.sync.dma_start(wv_sb[:], wv.ap().rearrange("(k p) d -> p k d", p=128))
            wo_sb = persist.tile([128, HPC, H], bf16, tag="wo")
            nc.sync.dma_start(wo_sb[:], wo.ap().rearrange("(c p) m -> p c m", p=128))

            qT_sb = persist.tile([128, HPC, S], bf16, tag="qT")
            kT_sb = persist.tile([128, HPC, S], bf16, tag="kT")
            v_sb = persist.tile([128, NK, DPC], bf16, tag="v")
            oT_sb = persist.tile([128, HPC, S], bf16, tag="oT")

            ones_sb = persist.tile([128, 1], bf16, tag="ones")
            nc.vector.memset(ones_sb[:], 1.0)

            for tau in range(NT):
                tsl = slice(tau * TBLK, (tau + 1) * TBLK)
                # --- phase 1: projections for this query block ---------
                xt = xt_pool.tile([128, HKT, TBLK], bf16, tag="xt")
                nc.sync.dma_start(
                    xt[:], xT.ap()[:, tsl].rearrange("(k p) t -> p k t", p=128))
                for w_sb, dst in ((wq_sb, qT_sb), (wk_sb, kT_sb)):
                    for h in range(HPC):
                        ps = ps_work.tile([128, TBLK], fp32, tag="ps")
                        for hk in range(HKT):
                            nc.tensor.matmul(
                                ps[:],
                                lhsT=w_sb[:, hk, h * HD:(h + 1) * HD],
                                rhs=xt[:, hk, :],
                                start=(hk == 0), stop=(hk == HKT - 1))
                        nc.any.tensor_copy(out=dst[:, h, tsl], in_=ps[:])
                for tb_local in range(TBLK // KBLK):
                    ps = ps_work.tile([128, TBLK], fp32, tag="ps")
                    for hk in range(HKT):
                        nc.tensor.matmul(
                            ps[:],
                            lhsT=xt[:, hk, tb_local * KBLK:(tb_local + 1) * KBLK],
                            rhs=wv_sb[:, hk, :],
                            start=(hk == 0), stop=(hk == HKT - 1))
                    nc.any.tensor_copy(
                        out=v_sb[:, tau * (TBLK // KBLK) + tb_local, :], in_=ps[:])

                # --- phase 2: attention for this query block -----------
                blocks = pattern[tau]
                mask_tiles = {}
                for Tb, partial in blocks:
                    if partial:
                        mt = mask_pool.tile([128, TBLK], bf16, tag="mask")
                        nc.sync.dma_start(
                            mt[:], maskT.ap()[Tb * KBLK:(Tb + 1) * KBLK, tsl])
                        mask_tiles[Tb] = mt
                for h in range(HPC):
                    od = ps_acc.tile([128, TBLK], fp32, tag="od")
                    dn = ps_den.tile([1, TBLK], fp32, tag="dn")
                    for i, (Tb, partial) in enumerate(blocks):
                        sp = ps_work.tile([128, TBLK], fp32, tag="ps")
                        nc.tensor.matmul(
                            sp[:],
                            lhsT=kT_sb[:, h, Tb * KBLK:(Tb + 1) * KBLK],
                            rhs=qT_sb[:, h, tsl],
                            start=True, stop=True)
                        e = e_pool.tile([128, TBLK], bf16, tag="e")
                        nc.scalar.activation(out=e[:], in_=sp[:], func=Exp,
                                             scale=inv_sqrt_hd)
                        if partial:
                            nc.vector.tensor_mul(e[:], e[:], mask_tiles[Tb][:])
                        last = i == len(blocks) - 1
                        nc.tensor.matmul(dn[:], lhsT=ones_sb[:], rhs=e[:],
                                         start=(i == 0), stop=last)
                        nc.tensor.matmul(
                            od[:],
                            lhsT=v_sb[:, Tb, h * HD:(h + 1) * HD],
                            rhs=e[:],
                            start=(i == 0), stop=last)
                    r = small_pool.tile([1, TBLK], fp32, tag="r")
                    nc.vector.reciprocal(r[:], dn[:])
                    R = small_pool.tile([128, TBLK], fp32, tag="R")
                    nc.gpsimd.partition_broadcast(R[:], r[:])
                    nc.vector.tensor_mul(oT_sb[:, h, tsl], od[:], R[:])

            # --- phase 3: output projection (partial over this core's
            # 512 rows of Wo; host sums partials across head-groups) ----
            for tt in range(S // 128):
                for mb in range(H // TBLK):
                    ps = ps_work.tile([128, TBLK], fp32, tag="ps")
                    for h in range(HPC):
                        nc.tensor.matmul(
                            ps[:],
                            lhsT=oT_sb[:, h, tt * 128:(tt + 1) * 128],
                            rhs=wo_sb[:, h, mb * TBLK:(mb + 1) * TBLK],
                            start=(h == 0), stop=(h == HPC - 1))
                    osb = out_pool.tile([128, TBLK], fp32, tag="osb")
                    nc.any.tensor_copy(out=osb[:], in_=ps[:])
                    nc.sync.dma_start(
                        out.ap()[tt * 128:(tt + 1) * 128,
                                 mb * TBLK:(mb + 1) * TBLK],
                        osb[:])

    nc.compile()
    return nc


def _classify(mask):
    """Per 128x512 block of mask^T: skip / full / partial, unioned over
    batches.  Returns the pattern tuple, or None if some row is fully
    masked (degenerate -- reference gives uniform weights there)."""
    if not mask.any(axis=2).all():
        return None
    pattern = []
    for tau in range(NT):
        blocks = []
        for Tb in range(NK):
            # block of mask^T[Tb*128:(Tb+1)*128, tau*512:(tau+1)*512]
            # == mask[:, tau*512:(tau+1)*512, Tb*128:(Tb+1)*128]
            blk = mask[:, tau * TBLK:(tau + 1) * TBLK,
                       Tb * KBLK:(Tb + 1) * KBLK]
            if not blk.any():
                continue
            blocks.append((Tb, not blk.all()))
        pattern.append(tuple(blocks))
    return tuple(pattern)


def _reference_fallback(x, mask, Wq, Wk, Wv, Wo):
    out = np.empty((B, S, H), np.float32)
    for b in range(B):
        q = (x[b] @ Wq).reshape(S, NH, HD).transpose(1, 0, 2)
        k = (x[b] @ Wk).reshape(S, NH, HD).transpose(1, 0, 2)
        v = (x[b] @ Wv).reshape(S, NH, HD).transpose(1, 0, 2)
        s = np.einsum("htd,hTd->htT", q, k) / np.sqrt(HD)
        s = np.where(mask[b][None], s, -1e10)
        s -= s.max(-1, keepdims=True)
        w = np.exp(s)
        w /= w.sum(-1, keepdims=True)
        o = np.einsum("htT,hTd->htd", w, v)
        out[b] = o.transpose(1, 0, 2).reshape(S, NH * HD) @ Wo
    return out


def kernel(x, mask, Wq, Wk, Wv, Wo):
    x = np.asarray(x, np.float32)
    mask = np.asarray(mask).astype(bool)
    Wq = np.asarray(Wq, np.float32)
    Wk = np.asarray(Wk, np.float32)
    Wv = np.asarray(Wv, np.float32)
    Wo = np.asarray(Wo, np.float32)
    assert x.shape == (B, S, H) and mask.shape == (B, S, S)

    pattern = _classify(mask)
    if pattern is None:
        return _reference_fallback(x, mask, Wq, Wk, Wv, Wo)

    if pattern not in _kernel_cache:
        _kernel_cache[pattern] = _build(pattern)
    nc = _kernel_cache[pattern]

    xT_b = [np.ascontiguousarray(x[b].T).astype(_BF16) for b in range(B)]
    maskT_b = [np.ascontiguousarray(mask[b].T).astype(_BF16) for b in range(B)]
    wq_g = [np.ascontiguousarray(Wq[:, g * DPC:(g + 1) * DPC]).astype(_BF16)
            for g in range(GROUPS)]
    wk_g = [np.ascontiguousarray(Wk[:, g * DPC:(g + 1) * DPC]).astype(_BF16)
            for g in range(GROUPS)]
    wv_g = [np.ascontiguousarray(Wv[:, g * DPC:(g + 1) * DPC]).astype(_BF16)
            for g in range(GROUPS)]
    wo_g = [np.ascontiguousarray(Wo[g * DPC:(g + 1) * DPC, :]).astype(_BF16)
            for g in range(GROUPS)]

    in_maps = []
    for i in range(N_CORES):
        b, g = divmod(i, GROUPS)
        in_maps.append({
            "xT": xT_b[b], "maskT": maskT_b[b],
            "wq": wq_g[g], "wk": wk_g[g], "wv": wv_g[g], "wo": wo_g[g],
        })

    from concourse.bass_utils import run_bass_kernel_spmd
    res = run_bass_kernel_spmd(nc, in_maps, core_ids=list(range(N_CORES)))

    out = np.zeros((B, S, H), np.float32)
    for i in range(N_CORES):
        b = i // GROUPS
        out[b] += res.results[i]["out"]
    return out


# revision 6
# speedup vs baseline: 1.0172x; 1.0172x over previous
"""Multi-head attention (B=2, S=2048, H=2048, NH=16, HD=128) on 8 trn2 cores.

Sharding: core i -> (batch b = i // 4, head-group g = i % 4, 4 heads each).
Each core computes q/k/v projections for its 4 heads, causal-masked
attention, and a partial output projection against its 512-row slice of
Wo.  The host sums the 4 partial outputs per batch.

Layout strategy (everything K-major so no on-chip transposes are needed):
  - host ships x^T (per batch) in bf16; projections compute q^T/k^T
    [d, t] via lhsT=W, rhs=x^T, and v [T, d] via lhsT=x^T, rhs=Wv.
  - scores^T [T, t] = (k^T).T @ q^T; exp on ACT (no max subtraction --
    scores are O(6) here, exp is safe in fp32); runtime mask applied
    multiplicatively AFTER exp (so softmax denominators stay exact).
  - denominator via ones-vector matmul (partition-dim reduce on PE);
    o^T [d, t] = v.T @ e accumulates in PSUM; normalized by broadcast
    reciprocal on the way out to SBUF.
  - final: out[t, m] = (o^T).T @ Wo_rows, accumulated over the 4 heads.

The mask is inspected on the host and the kernel is specialized per
128x512 block: skip (all False), full (all True), or partial (loads the
mask tile and multiplies).  For the causal mask this halves attention
FLOPs; for an all-ones mask it becomes a dense kernel automatically.
"""

import math

import numpy as np
import ml_dtypes

B, S, H, NH, HD = 2, 2048, 2048, 16, 128
N_CORES = 8
GROUPS = 4                # head-groups (cores per batch)
HPC = NH // GROUPS        # heads per core = 4
DPC = HPC * HD            # head dims per core = 512
TBLK = 512                # query-block width (matmul moving dim)
KBLK = 128                # key-block width (matmul contraction dim)
NT = S // TBLK            # 4 query blocks
NK = S // KBLK            # 16 key blocks
HKT = H // 128            # 16 contraction tiles over hidden dim
HKC = 4                   # contraction chunks per DMA (so loads pipeline)

_BF16 = ml_dtypes.bfloat16

_kernel_cache = {}


def _runs(blocks):
    """Group the partial blocks of one query block into contiguous Tb
    runs so each run loads with a single DMA."""
    runs = []
    for Tb, partial in blocks:
        if not partial:
            continue
        if runs and runs[-1][-1] == Tb - 1:
            runs[-1].append(Tb)
        else:
            runs.append([Tb])
    return runs


def _build(pattern):
    """Compile the SPMD program for a given mask block pattern.

    pattern: tuple over query-block tau of tuples of (Tb, partial) pairs,
    ascending in Tb, listing key blocks that have any visible entry.
    """
    import concourse.bass as bass  # noqa: F401
    import concourse.tile as tile
    from concourse import bacc, mybir

    fp32 = mybir.dt.float32
    bf16 = mybir.dt.bfloat16
    Exp = mybir.ActivationFunctionType.Exp
    inv_sqrt_hd = 1.0 / math.sqrt(HD)

    all_runs = [_runs(blocks) for blocks in pattern]
    max_run_len = max((len(r) for runs in all_runs for r in runs), default=1)
    max_runs = max((len(runs) for runs in all_runs), default=1)

    nc = bacc.Bacc("TRN2", target_bir_lowering=False, debug=False,
                   num_devices=N_CORES)
    xT = nc.dram_tensor("xT", [H, S], bf16, kind="ExternalInput")
    wq = nc.dram_tensor("wq", [H, DPC], bf16, kind="ExternalInput")
    wk = nc.dram_tensor("wk", [H, DPC], bf16, kind="ExternalInput")
    wv = nc.dram_tensor("wv", [H, DPC], bf16, kind="ExternalInput")
    wo = nc.dram_tensor("wo", [DPC, H], bf16, kind="ExternalInput")
    maskT = nc.dram_tensor("maskT", [S, S], bf16, kind="ExternalInput")
    out = nc.dram_tensor("out", [S, H], fp32, kind="ExternalOutput")

    # round-robin DMA issue across idle queue engines (never tensor: its
    # sequencer must stay dedicated to the matmul stream)
    def dma_engines():
        while True:
            yield nc.sync
            yield nc.gpsimd
            yield nc.scalar
    dmae = dma_engines()

    n_chunks = HKT // HKC  # 4

    with tile.TileContext(nc) as tc:
        with (
            tc.tile_pool(name="persist", bufs=1) as persist,
            tc.tile_pool(name="xt", bufs=6) as xt_pool,
            tc.tile_pool(name="masks", bufs=max(2 * max_runs, 2)) as mask_pool,
            tc.tile_pool(name="e", bufs=8) as e_pool,
            tc.tile_pool(name="outsb", bufs=2) as out_pool,
            tc.tile_pool(name="rp", bufs=2) as r_pool,
            tc.tile_pool(name="Rp", bufs=2) as R_pool,
            tc.tile_pool(name="ps_work", bufs=4, space="PSUM") as ps_work,
            tc.tile_pool(name="ps_acc", bufs=2, space="PSUM") as ps_acc,
            tc.tile_pool(name="ps_den", bufs=2, space="PSUM") as ps_den,
        ):
            # --- persistent SBUF tensors, loaded in chunks across queues
            w_sbs = {}
            for name, dram in (("wq", wq), ("wk", wk), ("wv", wv)):
                chunks = []
                for c in range(n_chunks):
                    t = persist.tile([128, HKC, DPC], bf16, tag=f"{name}{c}")
                    nc_ = next(dmae)
                    nc_.dma_start(
                        t[:],
                        dram.ap()[c * HKC * 128:(c + 1) * HKC * 128, :]
                        .rearrange("(k p) d -> p k d", p=128))
                    chunks.append(t)
                w_sbs[name] = chunks

            qT_sb = persist.tile([128, HPC, S], bf16, tag="qT")
            kT_sb = persist.tile([128, HPC, S], bf16, tag="kT")
            v_sb = persist.tile([128, NK, DPC], bf16, tag="v")
            oT_sb = persist.tile([128, HPC, S], bf16, tag="oT")

            ones_sb = persist.tile([128, 1], bf16, tag="ones")
            nc.vector.memset(ones_sb[:], 1.0)

            def w_chunk(name, hk):
                return w_sbs[name][hk // HKC][:, hk % HKC, :]

            for tau in range(NT):
                tsl = slice(tau * TBLK, (tau + 1) * TBLK)
                # --- phase 1: projections for this query block ---------
                xts = []
                for c in range(n_chunks):
                    t = xt_pool.tile([128, HKC, TBLK], bf16, tag="xt")
                    nc_ = next(dmae)
                    nc_.dma_start(
                        t[:],
                        xT.ap()[c * HKC * 128:(c + 1) * HKC * 128, tsl]
                        .rearrange("(k p) t -> p k t", p=128))
                    xts.append(t)

                def xt_chunk(hk):
                    return xts[hk // HKC][:, hk % HKC, :]

                for wname, dst in (("wq", qT_sb), ("wk", kT_sb)):
                    for h in range(HPC):
                        ps = ps_work.tile([128, TBLK], fp32, tag="ps")
                        for hk in range(HKT):
                            nc.tensor.matmul(
                                ps[:],
                                lhsT=w_chunk(wname, hk)[:, h * HD:(h + 1) * HD],
                                rhs=xt_chunk(hk),
                                start=(hk == 0), stop=(hk == HKT - 1))
                        nc.vector.tensor_copy(out=dst[:, h, tsl], in_=ps[:])
                for tb_local in range(TBLK // KBLK):
                    ps = ps_work.tile([128, TBLK], fp32, tag="ps")
                    for hk in range(HKT):
                        nc.tensor.matmul(
                            ps[:],
                            lhsT=xt_chunk(hk)[:, tb_local * KBLK:(tb_local + 1) * KBLK],
                            rhs=w_chunk("wv", hk),
                            start=(hk == 0), stop=(hk == HKT - 1))
                    nc.vector.tensor_copy(
                        out=v_sb[:, tau * (TBLK // KBLK) + tb_local, :], in_=ps[:])

                # --- phase 2: attention for this query block -----------
                blocks = pattern[tau]
                mask_tiles = {}
                for run in all_runs[tau]:
                    mt = mask_pool.tile([128, max_run_len, TBLK], bf16,
                                        tag="mask")
                    nc_ = next(dmae)
                    nc_.dma_start(
                        mt[:, :len(run), :],
                        maskT.ap()[run[0] * KBLK:(run[-1] + 1) * KBLK, tsl]
                        .rearrange("(k p) t -> p k t", p=128))
                    for j, Tb in enumerate(run):
                        mask_tiles[Tb] = mt[:, j, :]
                for h in range(HPC):
                    od = ps_acc.tile([128, TBLK], fp32, tag="od")
                    dn = ps_den.tile([1, TBLK], fp32, tag="dn")
                    for i, (Tb, partial) in enumerate(blocks):
                        sp = ps_work.tile([128, TBLK], fp32, tag="ps")
                        nc.tensor.matmul(
                            sp[:],
                            lhsT=kT_sb[:, h, Tb * KBLK:(Tb + 1) * KBLK],
                            rhs=qT_sb[:, h, tsl],
                            start=True, stop=True)
                        e = e_pool.tile([128, TBLK], bf16, tag="e")
                        nc.scalar.activation(out=e[:], in_=sp[:], func=Exp,
                                             scale=inv_sqrt_hd)
                        if partial:
                            nc.vector.tensor_mul(e[:], e[:], mask_tiles[Tb])
                        last = i == len(blocks) - 1
                        nc.tensor.matmul(dn[:], lhsT=ones_sb[:], rhs=e[:],
                                         start=(i == 0), stop=last)
                        nc.tensor.matmul(
                            od[:],
                            lhsT=v_sb[:, Tb, h * HD:(h + 1) * HD],
                            rhs=e[:],
                            start=(i == 0), stop=last)
                    r = r_pool.tile([1, TBLK], fp32, tag="r")
                    nc.vector.reciprocal_approx_fast(out=r[:], in_=dn[:])
                    R = R_pool.tile([128, TBLK], fp32, tag="R")
                    nc.gpsimd.partition_broadcast(R[:], r[:])
                    nc.vector.tensor_mul(oT_sb[:, h, tsl], od[:], R[:])

            # --- phase 3: output projection (partial over this core's
            # 512 rows of Wo; host sums partials across head-groups) ----
            wo_sb = persist.tile([128, HPC, H], bf16, tag="wo")
            nc.sync.dma_start(wo_sb[:], wo.ap().rearrange("(c p) m -> p c m", p=128))
            for tt in range(S // 128):
                for half in range(2):
                    osb = out_pool.tile([128, H // 2], fp32, tag="osb")
                    for mbl in range(H // TBLK // 2):
                        mb = half * (H // TBLK // 2) + mbl
                        ps = ps_work.tile([128, TBLK], fp32, tag="ps")
                        for h in range(HPC):
                            nc.tensor.matmul(
                                ps[:],
                                lhsT=oT_sb[:, h, tt * 128:(tt + 1) * 128],
                                rhs=wo_sb[:, h, mb * TBLK:(mb + 1) * TBLK],
                                start=(h == 0), stop=(h == HPC - 1))
                        nc.vector.tensor_copy(
                            out=osb[:, mbl * TBLK:(mbl + 1) * TBLK], in_=ps[:])
                    nc_ = next(dmae)
                    nc_.dma_start(
                        out.ap()[tt * 128:(tt + 1) * 128,
                                 half * (H // 2):(half + 1) * (H // 2)],
                        osb[:])

    nc.compile()
    return nc


def _classify(mask):
    """Per 128x512 block of mask^T: skip / full / partial, unioned over
    batches.  Returns the pattern tuple, or None if some row is fully
    masked (degenerate -- reference gives uniform weights there)."""
    if not mask.any(axis=2).all():
        return None
    pattern = []
    for tau in range(NT):
        blocks = []
        for Tb in range(NK):
            # block of mask^T[Tb*128:(Tb+1)*128, tau*512:(tau+1)*512]
            # == mask[:, tau*512:(tau+1)*512, Tb*128:(Tb+1)*128]
            blk = mask[:, tau * TBLK:(tau + 1) * TBLK,
                       Tb * KBLK:(Tb + 1) * KBLK]
            if not blk.any():
                continue
            blocks.append((Tb, not blk.all()))
        pattern.append(tuple(blocks))
    return tuple(pattern)


def _reference_fallback(x, mask, Wq, Wk, Wv, Wo):
    out = np.empty((B, S, H), np.float32)
    for b in range(B):
        q = (x[b] @ Wq).reshape(S, NH, HD).transpose(1, 0, 2)
        k = (x[b] @ Wk).reshape(S, NH, HD).transpose(1, 0, 2)
        v = (x[b] @ Wv).reshape(S, NH, HD).transpose(1, 0, 2)
        s = np.einsum("htd,hTd->htT", q, k) / np.sqrt(HD)
        s = np.where(mask[b][None], s, -1e10)
        s -= s.max(-1, keepdims=True)
        w = np.exp(s)
        w /= w.sum(-1, keepdims=True)
        o = np.einsum("htT,hTd->htd", w, v)
        out[b] = o.transpose(1, 0, 2).reshape(S, NH * HD) @ Wo
    return out


def kernel(x, mask, Wq, Wk, Wv, Wo):
    x = np.asarray(x, np.float32)
    mask = np.asarray(mask).astype(bool)
    Wq = np.asarray(Wq, np.float32)
    Wk = np.asarray(Wk, np.float32)
    Wv = np.asarray(Wv, np.float32)
    Wo = np.asarray(Wo, np.float32)
    assert x.shape == (B, S, H) and mask.shape == (B, S, S)

    pattern = _classify(mask)
    if pattern is None:
        return _reference_fallback(x, mask, Wq, Wk, Wv, Wo)

    if pattern not in _kernel_cache:
        _kernel_cache[pattern] = _build(pattern)
    nc = _kernel_cache[pattern]

    xT_b = [np.ascontiguousarray(x[b].T).astype(_BF16) for b in range(B)]
    maskT_b = [np.ascontiguousarray(mask[b].T).astype(_BF16) for b in range(B)]
    wq_g = [np.ascontiguousarray(Wq[:, g * DPC:(g + 1) * DPC]).astype(_BF16)
            for g in range(GROUPS)]
    wk_g = [np.ascontiguousarray(Wk[:, g * DPC:(g + 1) * DPC]).astype(_BF16)
            for g in range(GROUPS)]
    wv_g = [np.ascontiguousarray(Wv[:, g * DPC:(g + 1) * DPC]).astype(_BF16)
            for g in range(GROUPS)]
    wo_g = [np.ascontiguousarray(Wo[g * DPC:(g + 1) * DPC, :]).astype(_BF16)
            for g in range(GROUPS)]

    in_maps = []
    for i in range(N_CORES):
        b, g = divmod(i, GROUPS)
        in_maps.append({
            "xT": xT_b[b], "maskT": maskT_b[b],
            "wq": wq_g[g], "wk": wk_g[g], "wv": wv_g[g], "wo": wo_g[g],
        })

    from concourse.bass_utils import run_bass_kernel_spmd
    res = run_bass_kernel_spmd(nc, in_maps, core_ids=list(range(N_CORES)))

    out = np.zeros((B, S, H), np.float32)
    for i in range(N_CORES):
        b = i // GROUPS
        out[b] += res.results[i]["out"]
    return out


# revision 7
# speedup vs baseline: 1.1719x; 1.1521x over previous
"""Multi-head attention (B=2, S=2048, H=2048, NH=16, HD=128) on 8 trn2 cores.

Sharding: core i -> (batch b = i // 4, head-group g = i % 4, 4 heads each).
Each core computes q/k/v projections for its 4 heads, causal-masked
attention, and a partial output projection against its 512-row slice of
Wo.  The host sums the 4 partial outputs per batch.

Layout strategy (everything K-major so no on-chip transposes are needed):
  - host ships x^T (per batch) in bf16; projections compute q^T/k^T
    [d, t] via lhsT=W, rhs=x^T, and v [T, d] via lhsT=x^T, rhs=Wv.
  - scores^T [T, t] = (k^T).T @ q^T; exp on ACT (no max subtraction --
    scores are O(6) here, exp is safe in fp32); runtime mask applied
    multiplicatively AFTER exp (so softmax denominators stay exact).
  - denominator via ones-vector matmul (partition-dim reduce on PE);
    o^T [d, t] = v.T @ e accumulates in PSUM; normalized by broadcast
    reciprocal on the way out to SBUF.
  - final: out[t, m] = (o^T).T @ Wo_rows, accumulated over the 4 heads.

The mask is inspected on the host and the kernel is specialized per
128x512 block: skip (all False), full (all True), or partial (loads the
mask tile and multiplies).  For the causal mask this halves attention
FLOPs; for an all-ones mask it becomes a dense kernel automatically.
"""

import math

import numpy as np
import ml_dtypes

B, S, H, NH, HD = 2, 2048, 2048, 16, 128
N_CORES = 8
GROUPS = 4                # head-groups (cores per batch)
HPC = NH // GROUPS        # heads per core = 4
DPC = HPC * HD            # head dims per core = 512
TBLK = 512                # query-block width (matmul moving dim)
KBLK = 128                # key-block width (matmul contraction dim)
NT = S // TBLK            # 4 query blocks
NK = S // KBLK            # 16 key blocks
HKT = H // 128            # 16 contraction tiles over hidden dim
HKC = 4                   # contraction chunks per DMA (so loads pipeline)

_BF16 = ml_dtypes.bfloat16

_kernel_cache = {}


def _runs(blocks):
    """Group the partial blocks of one query block into contiguous Tb
    runs so each run loads with a single DMA."""
    runs = []
    for Tb, partial in blocks:
        if not partial:
            continue
        if runs and runs[-1][-1] == Tb - 1:
            runs[-1].append(Tb)
        else:
            runs.append([Tb])
    return runs


def _build(pattern):
    """Compile the SPMD program for a given mask block pattern.

    pattern: tuple over query-block tau of tuples of (Tb, partial) pairs,
    ascending in Tb, listing key blocks that have any visible entry.
    """
    import concourse.bass as bass  # noqa: F401
    import concourse.tile as tile
    from concourse import bacc, mybir

    fp32 = mybir.dt.float32
    bf16 = mybir.dt.bfloat16
    Exp = mybir.ActivationFunctionType.Exp
    inv_sqrt_hd = 1.0 / math.sqrt(HD)

    all_runs = [_runs(blocks) for blocks in pattern]
    max_run_len = max((len(r) for runs in all_runs for r in runs), default=1)
    max_runs = max((len(runs) for runs in all_runs), default=1)

    nc = bacc.Bacc("TRN2", target_bir_lowering=False, debug=False,
                   num_devices=N_CORES)
    xT = nc.dram_tensor("xT", [H, S], bf16, kind="ExternalInput")
    wq = nc.dram_tensor("wq", [H, DPC], bf16, kind="ExternalInput")
    wk = nc.dram_tensor("wk", [H, DPC], bf16, kind="ExternalInput")
    wv = nc.dram_tensor("wv", [H, DPC], bf16, kind="ExternalInput")
    wo = nc.dram_tensor("wo", [DPC, H], bf16, kind="ExternalInput")
    maskT = nc.dram_tensor("maskT", [S, S], bf16, kind="ExternalInput")
    out = nc.dram_tensor("out", [S, H], fp32, kind="ExternalOutput")

    # Explicit DMA queue discipline: sync carries the latency-critical
    # steady-state loads (xT blocks, masks), gpsimd carries weights at
    # startup and output stores, scalar carries weights only at startup
    # (it runs the exps afterwards).  Never tensor: its sequencer must
    # stay dedicated to the matmul stream.
    n_chunks = HKT // HKC  # 4

    with tile.TileContext(nc) as tc:
        with (
            tc.tile_pool(name="persist", bufs=1) as persist,
            tc.tile_pool(name="xt", bufs=6) as xt_pool,
            tc.tile_pool(name="masks", bufs=max(2 * max_runs, 2)) as mask_pool,
            tc.tile_pool(name="e", bufs=8) as e_pool,
            tc.tile_pool(name="outsb", bufs=2) as out_pool,
            tc.tile_pool(name="rp", bufs=2) as r_pool,
            tc.tile_pool(name="Rp", bufs=2) as R_pool,
            tc.tile_pool(name="ps_work", bufs=4, space="PSUM") as ps_work,
            tc.tile_pool(name="ps_acc", bufs=2, space="PSUM") as ps_acc,
            tc.tile_pool(name="ps_den", bufs=2, space="PSUM") as ps_den,
        ):
            # --- persistent SBUF tensors, loaded in chunks across queues
            w_sbs = {}
            for name, dram, eng in (("wq", wq, nc.gpsimd), ("wk", wk, nc.gpsimd),
                                    ("wv", wv, nc.scalar)):
                chunks = []
                for c in range(n_chunks):
                    t = persist.tile([128, HKC, DPC], bf16, tag=f"{name}{c}")
                    eng.dma_start(
                        t[:],
                        dram.ap()[c * HKC * 128:(c + 1) * HKC * 128, :]
                        .rearrange("(k p) d -> p k d", p=128))
                    chunks.append(t)
                w_sbs[name] = chunks
            wo_sb = persist.tile([128, HPC, H], bf16, tag="wo")
            nc.scalar.dma_start(
                wo_sb[:], wo.ap().rearrange("(c p) m -> p c m", p=128))

            qT_sb = persist.tile([128, HPC, S], bf16, tag="qT")
            kT_sb = persist.tile([128, HPC, S], bf16, tag="kT")
            v_sb = persist.tile([128, NK, DPC], bf16, tag="v")
            oT_sb = persist.tile([128, HPC, S], bf16, tag="oT")

            ones_sb = persist.tile([128, 1], bf16, tag="ones")
            nc.vector.memset(ones_sb[:], 1.0)

            def w_chunk(name, hk):
                return w_sbs[name][hk // HKC][:, hk % HKC, :]

            for tau in range(NT):
                tsl = slice(tau * TBLK, (tau + 1) * TBLK)
                # --- phase 1: projections for this query block ---------
                xts = []
                for c in range(n_chunks):
                    t = xt_pool.tile([128, HKC, TBLK], bf16, tag="xt")
                    nc.sync.dma_start(
                        t[:],
                        xT.ap()[c * HKC * 128:(c + 1) * HKC * 128, tsl]
                        .rearrange("(k p) t -> p k t", p=128))
                    xts.append(t)

                def xt_chunk(hk):
                    return xts[hk // HKC][:, hk % HKC, :]

                for wname, dst in (("wq", qT_sb), ("wk", kT_sb)):
                    for h in range(HPC):
                        ps = ps_work.tile([128, TBLK], fp32, tag="ps")
                        for hk in range(HKT):
                            nc.tensor.matmul(
                                ps[:],
                                lhsT=w_chunk(wname, hk)[:, h * HD:(h + 1) * HD],
                                rhs=xt_chunk(hk),
                                start=(hk == 0), stop=(hk == HKT - 1))
                        nc.vector.tensor_copy(out=dst[:, h, tsl], in_=ps[:])
                for tb_local in range(TBLK // KBLK):
                    ps = ps_work.tile([128, TBLK], fp32, tag="ps")
                    for hk in range(HKT):
                        nc.tensor.matmul(
                            ps[:],
                            lhsT=xt_chunk(hk)[:, tb_local * KBLK:(tb_local + 1) * KBLK],
                            rhs=w_chunk("wv", hk),
                            start=(hk == 0), stop=(hk == HKT - 1))
                    nc.vector.tensor_copy(
                        out=v_sb[:, tau * (TBLK // KBLK) + tb_local, :], in_=ps[:])

                # --- phase 2: attention for this query block -----------
                blocks = pattern[tau]
                mask_tiles = {}
                for run in all_runs[tau]:
                    mt = mask_pool.tile([128, max_run_len, TBLK], bf16,
                                        tag="mask")
                    nc.sync.dma_start(
                        mt[:, :len(run), :],
                        maskT.ap()[run[0] * KBLK:(run[-1] + 1) * KBLK, tsl]
                        .rearrange("(k p) t -> p k t", p=128))
                    for j, Tb in enumerate(run):
                        mask_tiles[Tb] = mt[:, j, :]
                for h in range(HPC):
                    od = ps_acc.tile([128, TBLK], fp32, tag="od")
                    dn = ps_den.tile([1, TBLK], fp32, tag="dn")
                    for i, (Tb, partial) in enumerate(blocks):
                        sp = ps_work.tile([128, TBLK], fp32, tag="ps")
                        nc.tensor.matmul(
                            sp[:],
                            lhsT=kT_sb[:, h, Tb * KBLK:(Tb + 1) * KBLK],
                            rhs=qT_sb[:, h, tsl],
                            start=True, stop=True)
                        e = e_pool.tile([128, TBLK], bf16, tag="e")
                        nc.scalar.activation(out=e[:], in_=sp[:], func=Exp,
                                             scale=inv_sqrt_hd)
                        if partial:
                            nc.vector.tensor_mul(e[:], e[:], mask_tiles[Tb])
                        last = i == len(blocks) - 1
                        nc.tensor.matmul(dn[:], lhsT=ones_sb[:], rhs=e[:],
                                         start=(i == 0), stop=last)
                        nc.tensor.matmul(
                            od[:],
                            lhsT=v_sb[:, Tb, h * HD:(h + 1) * HD],
                            rhs=e[:],
                            start=(i == 0), stop=last)
                    r = r_pool.tile([1, TBLK], fp32, tag="r")
                    nc.vector.reciprocal_approx_fast(out=r[:], in_=dn[:])
                    R = R_pool.tile([128, TBLK], fp32, tag="R")
                    nc.gpsimd.partition_broadcast(R[:], r[:])
                    nc.vector.tensor_mul(oT_sb[:, h, tsl], od[:], R[:])

                # --- phase 3 for this query block: output projection
                # (partial over this core's 512 rows of Wo; host sums
                # partials across head-groups). Interleaved per tau so the
                # 16 MB of output stores spread across the whole kernel. --
                for tt in range(tau * (TBLK // 128), (tau + 1) * (TBLK // 128)):
                    for half in range(2):
                        osb = out_pool.tile([128, H // 2], fp32, tag="osb")
                        for mbl in range(H // TBLK // 2):
                            mb = half * (H // TBLK // 2) + mbl
                            ps = ps_work.tile([128, TBLK], fp32, tag="ps")
                            for h in range(HPC):
                                nc.tensor.matmul(
                                    ps[:],
                                    lhsT=oT_sb[:, h, tt * 128:(tt + 1) * 128],
                                    rhs=wo_sb[:, h, mb * TBLK:(mb + 1) * TBLK],
                                    start=(h == 0), stop=(h == HPC - 1))
                            nc.vector.tensor_copy(
                                out=osb[:, mbl * TBLK:(mbl + 1) * TBLK], in_=ps[:])
                        nc.gpsimd.dma_start(
                            out.ap()[tt * 128:(tt + 1) * 128,
                                     half * (H // 2):(half + 1) * (H // 2)],
                            osb[:])

    nc.compile()
    return nc


def _classify(mask):
    """Per 128x512 block of mask^T: skip / full / partial, unioned over
    batches.  Returns the pattern tuple, or None if some row is fully
    masked (degenerate -- reference gives uniform weights there)."""
    if not mask.any(axis=2).all():
        return None
    pattern = []
    for tau in range(NT):
        blocks = []
        for Tb in range(NK):
            # block of mask^T[Tb*128:(Tb+1)*128, tau*512:(tau+1)*512]
            # == mask[:, tau*512:(tau+1)*512, Tb*128:(Tb+1)*128]
            blk = mask[:, tau * TBLK:(tau + 1) * TBLK,
                       Tb * KBLK:(Tb + 1) * KBLK]
            if not blk.any():
                continue
            blocks.append((Tb, not blk.all()))
        pattern.append(tuple(blocks))
    return tuple(pattern)


def _reference_fallback(x, mask, Wq, Wk, Wv, Wo):
    out = np.empty((B, S, H), np.float32)
    for b in range(B):
        q = (x[b] @ Wq).reshape(S, NH, HD).transpose(1, 0, 2)
        k = (x[b] @ Wk).reshape(S, NH, HD).transpose(1, 0, 2)
        v = (x[b] @ Wv).reshape(S, NH, HD).transpose(1, 0, 2)
        s = np.einsum("htd,hTd->htT", q, k) / np.sqrt(HD)
        s = np.where(mask[b][None], s, -1e10)
        s -= s.max(-1, keepdims=True)
        w = np.exp(s)
        w /= w.sum(-1, keepdims=True)
        o = np.einsum("htT,hTd->htd", w, v)
        out[b] = o.transpose(1, 0, 2).reshape(S, NH * HD) @ Wo
    return out


def kernel(x, mask, Wq, Wk, Wv, Wo):
    x = np.asarray(x, np.float32)
    mask = np.asarray(mask).astype(bool)
    Wq = np.asarray(Wq, np.float32)
    Wk = np.asarray(Wk, np.float32)
    Wv = np.asarray(Wv, np.float32)
    Wo = np.asarray(Wo, np.float32)
    assert x.shape == (B, S, H) and mask.shape == (B, S, S)

    pattern = _classify(mask)
    if pattern is None:
        return _reference_fallback(x, mask, Wq, Wk, Wv, Wo)

    if pattern not in _kernel_cache:
        _kernel_cache[pattern] = _build(pattern)
    nc = _kernel_cache[pattern]

    xT_b = [np.ascontiguousarray(x[b].T).astype(_BF16) for b in range(B)]
    maskT_b = [np.ascontiguousarray(mask[b].T).astype(_BF16) for b in range(B)]
    wq_g = [np.ascontiguousarray(Wq[:, g * DPC:(g + 1) * DPC]).astype(_BF16)
            for g in range(GROUPS)]
    wk_g = [np.ascontiguousarray(Wk[:, g * DPC:(g + 1) * DPC]).astype(_BF16)
            for g in range(GROUPS)]
    wv_g = [np.ascontiguousarray(Wv[:, g * DPC:(g + 1) * DPC]).astype(_BF16)
            for g in range(GROUPS)]
    wo_g = [np.ascontiguousarray(Wo[g * DPC:(g + 1) * DPC, :]).astype(_BF16)
            for g in range(GROUPS)]

    in_maps = []
    for i in range(N_CORES):
        b, g = divmod(i, GROUPS)
        in_maps.append({
            "xT": xT_b[b], "maskT": maskT_b[b],
            "wq": wq_g[g], "wk": wk_g[g], "wv": wv_g[g], "wo": wo_g[g],
        })

    from concourse.bass_utils import run_bass_kernel_spmd
    res = run_bass_kernel_spmd(nc, in_maps, core_ids=list(range(N_CORES)))

    out = np.zeros((B, S, H), np.float32)
    for i in range(N_CORES):
        b = i // GROUPS
        out[b] += res.results[i]["out"]
    return out


# revision 8
# speedup vs baseline: 1.2093x; 1.0319x over previous
"""Multi-head attention (B=2, S=2048, H=2048, NH=16, HD=128) on 8 trn2 cores.

Sharding: core i -> (batch b = i // 4, head-group g = i % 4, 4 heads each).
Each core computes q/k/v projections for its 4 heads, causal-masked
attention, and a partial output projection against its 512-row slice of
Wo.  The host sums the 4 partial outputs per batch.

Layout strategy (everything K-major so no on-chip transposes are needed):
  - host ships x^T (per batch) in bf16; projections compute q^T/k^T
    [d, t] via lhsT=W, rhs=x^T, and v [T, d] via lhsT=x^T, rhs=Wv.
  - scores^T [T, t] = (k^T).T @ q^T; exp on ACT (no max subtraction --
    scores are O(6) here, exp is safe in fp32); runtime mask applied
    multiplicatively AFTER exp (so softmax denominators stay exact).
  - denominator via ones-vector matmul (partition-dim reduce on PE);
    o^T [d, t] = v.T @ e accumulates in PSUM; normalized by broadcast
    reciprocal on the way out to SBUF.
  - final: out[t, m] = (o^T).T @ Wo_rows, accumulated over the 4 heads.

The mask is inspected on the host and the kernel is specialized per
128x512 block: skip (all False), full (all True), or partial (loads the
mask tile and multiplies).  For the causal mask this halves attention
FLOPs; for an all-ones mask it becomes a dense kernel automatically.
"""

import math

import numpy as np
import ml_dtypes

B, S, H, NH, HD = 2, 2048, 2048, 16, 128
N_CORES = 8
GROUPS = 4                # head-groups (cores per batch)
HPC = NH // GROUPS        # heads per core = 4
DPC = HPC * HD            # head dims per core = 512
TBLK = 512                # query-block width (matmul moving dim)
KBLK = 128                # key-block width (matmul contraction dim)
NT = S // TBLK            # 4 query blocks
NK = S // KBLK            # 16 key blocks
HKT = H // 128            # 16 contraction tiles over hidden dim
HKC = 4                   # contraction chunks per DMA (so loads pipeline)

_BF16 = ml_dtypes.bfloat16

_kernel_cache = {}


def _runs(blocks):
    """Group the partial blocks of one query block into contiguous Tb
    runs so each run loads with a single DMA."""
    runs = []
    for Tb, partial in blocks:
        if not partial:
            continue
        if runs and runs[-1][-1] == Tb - 1:
            runs[-1].append(Tb)
        else:
            runs.append([Tb])
    return runs


def _build(pattern):
    """Compile the SPMD program for a given mask block pattern.

    pattern: tuple over query-block tau of tuples of (Tb, partial) pairs,
    ascending in Tb, listing key blocks that have any visible entry.
    """
    import concourse.bass as bass  # noqa: F401
    import concourse.tile as tile
    from concourse import bacc, mybir

    fp32 = mybir.dt.float32
    bf16 = mybir.dt.bfloat16
    Exp = mybir.ActivationFunctionType.Exp
    inv_sqrt_hd = 1.0 / math.sqrt(HD)

    all_runs = [_runs(blocks) for blocks in pattern]
    max_run_len = max((len(r) for runs in all_runs for r in runs), default=1)
    max_runs = max((len(runs) for runs in all_runs), default=1)

    nc = bacc.Bacc("TRN2", target_bir_lowering=False, debug=False,
                   num_devices=N_CORES)
    xT = nc.dram_tensor("xT", [H, S], bf16, kind="ExternalInput")
    wq = nc.dram_tensor("wq", [H, DPC], bf16, kind="ExternalInput")
    wk = nc.dram_tensor("wk", [H, DPC], bf16, kind="ExternalInput")
    wv = nc.dram_tensor("wv", [H, DPC], bf16, kind="ExternalInput")
    wo = nc.dram_tensor("wo", [DPC, H], bf16, kind="ExternalInput")
    maskT = nc.dram_tensor("maskT", [S, S], bf16, kind="ExternalInput")
    out = nc.dram_tensor("out", [S, H], fp32, kind="ExternalOutput")

    # Explicit DMA queue discipline: sync carries the latency-critical
    # steady-state loads (xT blocks, masks), gpsimd carries weights at
    # startup and output stores, scalar carries weights only at startup
    # (it runs the exps afterwards).  Never tensor: its sequencer must
    # stay dedicated to the matmul stream.
    n_chunks = HKT // HKC  # 4

    with tile.TileContext(nc) as tc:
        with (
            tc.tile_pool(name="persist", bufs=1) as persist,
            tc.tile_pool(name="xt", bufs=6) as xt_pool,
            tc.tile_pool(name="masks", bufs=max(2 * max_runs, 2)) as mask_pool,
            tc.tile_pool(name="e", bufs=8) as e_pool,
            tc.tile_pool(name="outsb", bufs=2) as out_pool,
            tc.tile_pool(name="esum", bufs=3) as esum_pool,
            tc.tile_pool(name="rp", bufs=2) as r_pool,
            tc.tile_pool(name="Rp", bufs=2) as R_pool,
            tc.tile_pool(name="ps_work", bufs=4, space="PSUM") as ps_work,
            tc.tile_pool(name="ps_acc", bufs=2, space="PSUM") as ps_acc,
            tc.tile_pool(name="ps_den", bufs=2, space="PSUM") as ps_den,
        ):
            # --- persistent SBUF tensors, loaded in chunks across queues
            w_sbs = {}
            for name, dram, eng in (("wq", wq, nc.gpsimd), ("wk", wk, nc.gpsimd),
                                    ("wv", wv, nc.scalar)):
                chunks = []
                for c in range(n_chunks):
                    t = persist.tile([128, HKC, DPC], bf16, tag=f"{name}{c}")
                    eng.dma_start(
                        t[:],
                        dram.ap()[c * HKC * 128:(c + 1) * HKC * 128, :]
                        .rearrange("(k p) d -> p k d", p=128))
                    chunks.append(t)
                w_sbs[name] = chunks
            wo_sb = persist.tile([128, HPC, H], bf16, tag="wo")
            nc.scalar.dma_start(
                wo_sb[:], wo.ap().rearrange("(c p) m -> p c m", p=128))

            qT_sb = persist.tile([128, HPC, S], bf16, tag="qT")
            kT_sb = persist.tile([128, HPC, S], bf16, tag="kT")
            v_sb = persist.tile([128, NK, DPC], bf16, tag="v")
            oT_sb = persist.tile([128, HPC, S], bf16, tag="oT")

            ones32_sb = persist.tile([128, 1], fp32, tag="ones32")
            nc.vector.memset(ones32_sb[:], 1.0)

            def w_chunk(name, hk):
                return w_sbs[name][hk // HKC][:, hk % HKC, :]

            def emit_phase3(ptau):
                # output projection rows of query block ptau (partial over
                # this core's 512 rows of Wo; host sums the partials).
                for tt in range(ptau * (TBLK // 128), (ptau + 1) * (TBLK // 128)):
                    for half in range(2):
                        osb = out_pool.tile([128, H // 2], fp32, tag="osb")
                        for mbl in range(H // TBLK // 2):
                            mb = half * (H // TBLK // 2) + mbl
                            ps = ps_work.tile([128, TBLK], fp32, tag="ps")
                            for h in range(HPC):
                                nc.tensor.matmul(
                                    ps[:],
                                    lhsT=oT_sb[:, h, tt * 128:(tt + 1) * 128],
                                    rhs=wo_sb[:, h, mb * TBLK:(mb + 1) * TBLK],
                                    start=(h == 0), stop=(h == HPC - 1))
                            nc.vector.tensor_copy(
                                out=osb[:, mbl * TBLK:(mbl + 1) * TBLK], in_=ps[:])
                        nc.gpsimd.dma_start(
                            out.ap()[tt * 128:(tt + 1) * 128,
                                     half * (H // 2):(half + 1) * (H // 2)],
                            osb[:])

            for tau in range(NT):
                tsl = slice(tau * TBLK, (tau + 1) * TBLK)
                # --- phase 1: projections for this query block ---------
                xts = []
                for c in range(n_chunks):
                    t = xt_pool.tile([128, HKC, TBLK], bf16, tag="xt")
                    nc.sync.dma_start(
                        t[:],
                        xT.ap()[c * HKC * 128:(c + 1) * HKC * 128, tsl]
                        .rearrange("(k p) t -> p k t", p=128))
                    xts.append(t)

                def xt_chunk(hk):
                    return xts[hk // HKC][:, hk % HKC, :]

                for wname, dst in (("wq", qT_sb), ("wk", kT_sb)):
                    for h in range(HPC):
                        ps = ps_work.tile([128, TBLK], fp32, tag="ps")
                        for hk in range(HKT):
                            nc.tensor.matmul(
                                ps[:],
                                lhsT=w_chunk(wname, hk)[:, h * HD:(h + 1) * HD],
                                rhs=xt_chunk(hk),
                                start=(hk == 0), stop=(hk == HKT - 1))
                        nc.vector.tensor_copy(out=dst[:, h, tsl], in_=ps[:])
                for tb_local in range(TBLK // KBLK):
                    ps = ps_work.tile([128, TBLK], fp32, tag="ps")
                    for hk in range(HKT):
                        nc.tensor.matmul(
                            ps[:],
                            lhsT=xt_chunk(hk)[:, tb_local * KBLK:(tb_local + 1) * KBLK],
                            rhs=w_chunk("wv", hk),
                            start=(hk == 0), stop=(hk == HKT - 1))
                    nc.vector.tensor_copy(
                        out=v_sb[:, tau * (TBLK // KBLK) + tb_local, :], in_=ps[:])

                # --- phase 3 for the PREVIOUS query block, emitted here so
                # the PE reaches it long after its normalizes finished ----
                if tau > 0:
                    emit_phase3(tau - 1)

                # --- phase 2: attention for this query block -----------
                blocks = pattern[tau]
                mask_tiles = {}
                for run in all_runs[tau]:
                    mt = mask_pool.tile([128, max_run_len, TBLK], bf16,
                                        tag="mask")
                    nc.sync.dma_start(
                        mt[:, :len(run), :],
                        maskT.ap()[run[0] * KBLK:(run[-1] + 1) * KBLK, tsl]
                        .rearrange("(k p) t -> p k t", p=128))
                    for j, Tb in enumerate(run):
                        mask_tiles[Tb] = mt[:, j, :]
                for h in range(HPC):
                    od = ps_acc.tile([128, TBLK], fp32, tag="od")
                    esum = esum_pool.tile([128, TBLK], fp32, tag="esum")
                    for i, (Tb, partial) in enumerate(blocks):
                        sp = ps_work.tile([128, TBLK], fp32, tag="ps")
                        nc.tensor.matmul(
                            sp[:],
                            lhsT=kT_sb[:, h, Tb * KBLK:(Tb + 1) * KBLK],
                            rhs=qT_sb[:, h, tsl],
                            start=True, stop=True)
                        e = e_pool.tile([128, TBLK], bf16, tag="e")
                        nc.scalar.activation(out=e[:], in_=sp[:], func=Exp,
                                             scale=inv_sqrt_hd)
                        if partial:
                            nc.vector.tensor_mul(e[:], e[:], mask_tiles[Tb])
                        last = i == len(blocks) - 1
                        if i == 0:
                            nc.vector.tensor_copy(out=esum[:], in_=e[:])
                        else:
                            nc.vector.tensor_add(esum[:], esum[:], e[:])
                        nc.tensor.matmul(
                            od[:],
                            lhsT=v_sb[:, Tb, h * HD:(h + 1) * HD],
                            rhs=e[:],
                            start=(i == 0), stop=last)
                    dn = ps_den.tile([1, TBLK], fp32, tag="dn")
                    nc.tensor.matmul(dn[:], lhsT=ones32_sb[:], rhs=esum[:],
                                     start=True, stop=True)
                    r = r_pool.tile([1, TBLK], fp32, tag="r")
                    nc.vector.reciprocal_approx_fast(out=r[:], in_=dn[:])
                    R = R_pool.tile([128, TBLK], fp32, tag="R")
                    nc.gpsimd.partition_broadcast(R[:], r[:])
                    nc.vector.tensor_mul(oT_sb[:, h, tsl], od[:], R[:])

            emit_phase3(NT - 1)

    nc.compile()
    return nc


def _classify(mask):
    """Per 128x512 block of mask^T: skip / full / partial, unioned over
    batches.  Returns the pattern tuple, or None if some row is fully
    masked (degenerate -- reference gives uniform weights there)."""
    if not mask.any(axis=2).all():
        return None
    pattern = []
    for tau in range(NT):
        blocks = []
        for Tb in range(NK):
            # block of mask^T[Tb*128:(Tb+1)*128, tau*512:(tau+1)*512]
            # == mask[:, tau*512:(tau+1)*512, Tb*128:(Tb+1)*128]
            blk = mask[:, tau * TBLK:(tau + 1) * TBLK,
                       Tb * KBLK:(Tb + 1) * KBLK]
            if not blk.any():
                continue
            blocks.append((Tb, not blk.all()))
        pattern.append(tuple(blocks))
    return tuple(pattern)


def _reference_fallback(x, mask, Wq, Wk, Wv, Wo):
    out = np.empty((B, S, H), np.float32)
    for b in range(B):
        q = (x[b] @ Wq).reshape(S, NH, HD).transpose(1, 0, 2)
        k = (x[b] @ Wk).reshape(S, NH, HD).transpose(1, 0, 2)
        v = (x[b] @ Wv).reshape(S, NH, HD).transpose(1, 0, 2)
        s = np.einsum("htd,hTd->htT", q, k) / np.sqrt(HD)
        s = np.where(mask[b][None], s, -1e10)
        s -= s.max(-1, keepdims=True)
        w = np.exp(s)
        w /= w.sum(-1, keepdims=True)
        o = np.einsum("htT,hTd->htd", w, v)
        out[b] = o.transpose(1, 0, 2).reshape(S, NH * HD) @ Wo
    return out


def kernel(x, mask, Wq, Wk, Wv, Wo):
    x = np.asarray(x, np.float32)
    mask = np.asarray(mask).astype(bool)
    Wq = np.asarray(Wq, np.float32)
    Wk = np.asarray(Wk, np.float32)
    Wv = np.asarray(Wv, np.float32)
    Wo = np.asarray(Wo, np.float32)
    assert x.shape == (B, S, H) and mask.shape == (B, S, S)

    pattern = _classify(mask)
    if pattern is None:
        return _reference_fallback(x, mask, Wq, Wk, Wv, Wo)

    if pattern not in _kernel_cache:
        _kernel_cache[pattern] = _build(pattern)
    nc = _kernel_cache[pattern]

    xT_b = [np.ascontiguousarray(x[b].T).astype(_BF16) for b in range(B)]
    maskT_b = [np.ascontiguousarray(mask[b].T).astype(_BF16) for b in range(B)]
    wq_g = [np.ascontiguousarray(Wq[:, g * DPC:(g + 1) * DPC]).astype(_BF16)
            for g in range(GROUPS)]
    wk_g = [np.ascontiguousarray(Wk[:, g * DPC:(g + 1) * DPC]).astype(_BF16)
            for g in range(GROUPS)]
    wv_g = [np.ascontiguousarray(Wv[:, g * DPC:(g + 1) * DPC]).astype(_BF16)
            for g in range(GROUPS)]
    wo_g = [np.ascontiguousarray(Wo[g * DPC:(g + 1) * DPC, :]).astype(_BF16)
            for g in range(GROUPS)]

    in_maps = []
    for i in range(N_CORES):
        b, g = divmod(i, GROUPS)
        in_maps.append({
            "xT": xT_b[b], "maskT": maskT_b[b],
            "wq": wq_g[g], "wk": wk_g[g], "wv": wv_g[g], "wo": wo_g[g],
        })

    from concourse.bass_utils import run_bass_kernel_spmd
    res = run_bass_kernel_spmd(nc, in_maps, core_ids=list(range(N_CORES)))

    out = np.zeros((B, S, H), np.float32)
    for i in range(N_CORES):
        b = i // GROUPS
        out[b] += res.results[i]["out"]
    return out


# revision 9
# speedup vs baseline: 1.2556x; 1.0383x over previous
"""Multi-head attention (B=2, S=2048, H=2048, NH=16, HD=128) on 8 trn2 cores.

Sharding: core i -> (batch b = i // 4, head-group g = i % 4, 4 heads each).
Each core computes q/k/v projections for its 4 heads, causal-masked
attention, and a partial output projection against its 512-row slice of
Wo.  The host sums the 4 partial outputs per batch.

Layout strategy (everything K-major so no on-chip transposes are needed):
  - host ships x^T (per batch) in bf16; projections compute q^T/k^T
    [d, t] via lhsT=W, rhs=x^T, and v [T, d] via lhsT=x^T, rhs=Wv.
  - scores^T [T, t] = (k^T).T @ q^T; exp on ACT (no max subtraction --
    scores are O(6) here, exp is safe in fp32); runtime mask applied
    multiplicatively AFTER exp (so softmax denominators stay exact).
  - softmax denominators: e tiles accumulate on DVE into an fp32 esum,
    reduced across partitions with one ones-matmul per (head, block);
    o^T [d, t] = v.T @ e accumulates in PSUM; normalized by broadcast
    reciprocal on the way out to SBUF.
  - final: out[t, m] = (o^T).T @ Wo_rows, accumulated over the 4 heads.

The mask is inspected on the host and the kernel is specialized per
128x512 block: skip (all False), full (all True), or partial (loads the
mask tile and multiplies).  For the causal mask this halves attention
FLOPs; for an all-ones mask it becomes a dense kernel automatically.

Emission is software-pipelined: in query-block tau's slot we emit its
attention heads round-robin with the projections of tau+1 and deferred
output-projection rows, so the PE always has independent matmul work
while ACT grinds through the exps.
"""

import math

import numpy as np
import ml_dtypes

B, S, H, NH, HD = 2, 2048, 2048, 16, 128
N_CORES = 8
GROUPS = 4                # head-groups (cores per batch)
HPC = NH // GROUPS        # heads per core = 4
DPC = HPC * HD            # head dims per core = 512
TBLK = 512                # query-block width (matmul moving dim)
KBLK = 128                # key-block width (matmul contraction dim)
NT = S // TBLK            # 4 query blocks
NK = S // KBLK            # 16 key blocks
HKT = H // 128            # 16 contraction tiles over hidden dim
HKC = 4                   # contraction chunks per DMA (so loads pipeline)

_BF16 = ml_dtypes.bfloat16

_kernel_cache = {}


def _runs(blocks):
    """Group the partial blocks of one query block into contiguous Tb
    runs so each run loads with a single DMA."""
    runs = []
    for Tb, partial in blocks:
        if not partial:
            continue
        if runs and runs[-1][-1] == Tb - 1:
            runs[-1].append(Tb)
        else:
            runs.append([Tb])
    return runs


def _interleave(primary, fillers):
    """Round-robin: after primary unit i, its even share of fillers."""
    out = []
    n = max(len(primary), 1)
    for i, p in enumerate(primary):
        out.append(p)
        out.extend(fillers[i * len(fillers) // n:(i + 1) * len(fillers) // n])
    out.extend(fillers[len(primary) * len(fillers) // n:])
    return out


def _build(pattern):
    """Compile the SPMD program for a given mask block pattern.

    pattern: tuple over query-block tau of tuples of (Tb, partial) pairs,
    ascending in Tb, listing key blocks that have any visible entry.
    """
    import concourse.bass as bass  # noqa: F401
    import concourse.tile as tile
    from concourse import bacc, mybir

    fp32 = mybir.dt.float32
    bf16 = mybir.dt.bfloat16
    Exp = mybir.ActivationFunctionType.Exp
    inv_sqrt_hd = 1.0 / math.sqrt(HD)

    all_runs = [_runs(blocks) for blocks in pattern]
    max_run_len = max((len(r) for runs in all_runs for r in runs), default=1)
    max_runs = max((len(runs) for runs in all_runs), default=1)

    nc = bacc.Bacc("TRN2", target_bir_lowering=False, debug=False,
                   num_devices=N_CORES)
    xT = nc.dram_tensor("xT", [H, S], bf16, kind="ExternalInput")
    wq = nc.dram_tensor("wq", [H, DPC], bf16, kind="ExternalInput")
    wk = nc.dram_tensor("wk", [H, DPC], bf16, kind="ExternalInput")
    wv = nc.dram_tensor("wv", [H, DPC], bf16, kind="ExternalInput")
    wo = nc.dram_tensor("wo", [DPC, H], bf16, kind="ExternalInput")
    maskT = nc.dram_tensor("maskT", [S, S], bf16, kind="ExternalInput")
    out = nc.dram_tensor("out", [S, H], fp32, kind="ExternalOutput")

    n_chunks = HKT // HKC  # 4

    with tile.TileContext(nc) as tc:
        with (
            tc.tile_pool(name="persist", bufs=1) as persist,
            tc.tile_pool(name="xt", bufs=6) as xt_pool,
            tc.tile_pool(name="masks", bufs=max(2 * max_runs, 2)) as mask_pool,
            tc.tile_pool(name="e", bufs=8) as e_pool,
            tc.tile_pool(name="outsb", bufs=2) as out_pool,
            tc.tile_pool(name="esum", bufs=3) as esum_pool,
            tc.tile_pool(name="rp", bufs=2) as r_pool,
            tc.tile_pool(name="Rp", bufs=2) as R_pool,
            tc.tile_pool(name="ps_work", bufs=4, space="PSUM") as ps_work,
            tc.tile_pool(name="ps_acc", bufs=2, space="PSUM") as ps_acc,
            tc.tile_pool(name="ps_den", bufs=2, space="PSUM") as ps_den,
        ):
            # --- persistent SBUF tensors -------------------------------
            # DMA queue discipline: sync carries the latency-critical
            # steady loads (xT blocks, masks), gpsimd the weights at
            # startup plus output stores, scalar only wo (emitted late --
            # it queues behind the first exps, landing well before
            # phase 3 needs it).  Never tensor: its sequencer must stay
            # dedicated to the matmul stream.
            w_sbs = {}
            for name, dram in (("wq", wq), ("wk", wk), ("wv", wv)):
                chunks = []
                for c in range(n_chunks):
                    t = persist.tile([128, HKC, DPC], bf16, tag=f"{name}{c}")
                    nc.gpsimd.dma_start(
                        t[:],
                        dram.ap()[c * HKC * 128:(c + 1) * HKC * 128, :]
                        .rearrange("(k p) d -> p k d", p=128))
                    chunks.append(t)
                w_sbs[name] = chunks
            wo_sb = persist.tile([128, HPC, H], bf16, tag="wo")

            qT_sb = persist.tile([128, HPC, S], bf16, tag="qT")
            kT_sb = persist.tile([128, HPC, S], bf16, tag="kT")
            v_sb = persist.tile([128, NK, DPC], bf16, tag="v")
            oT_sb = persist.tile([128, HPC, S], bf16, tag="oT")

            ones32_sb = persist.tile([128, 1], fp32, tag="ones32")
            nc.vector.memset(ones32_sb[:], 1.0)

            def w_chunk(name, hk):
                return w_sbs[name][hk // HKC][:, hk % HKC, :]

            xts = {}

            def emit_xt_load(tau):
                tsl = slice(tau * TBLK, (tau + 1) * TBLK)
                xts[tau] = []
                for c in range(n_chunks):
                    t = xt_pool.tile([128, HKC, TBLK], bf16, tag="xt")
                    nc.sync.dma_start(
                        t[:],
                        xT.ap()[c * HKC * 128:(c + 1) * HKC * 128, tsl]
                        .rearrange("(k p) t -> p k t", p=128))
                    xts[tau].append(t)

            def xt_chunk(tau, hk):
                return xts[tau][hk // HKC][:, hk % HKC, :]

            def emit_qk_proj(tau, wname, h):
                tsl = slice(tau * TBLK, (tau + 1) * TBLK)
                dst = qT_sb if wname == "wq" else kT_sb
                ps = ps_work.tile([128, TBLK], fp32, tag="ps")
                for hk in range(HKT):
                    nc.tensor.matmul(
                        ps[:],
                        lhsT=w_chunk(wname, hk)[:, h * HD:(h + 1) * HD],
                        rhs=xt_chunk(tau, hk),
                        start=(hk == 0), stop=(hk == HKT - 1))
                nc.vector.tensor_copy(out=dst[:, h, tsl], in_=ps[:])

            def emit_v_proj(tau, tb_local):
                ps = ps_work.tile([128, TBLK], fp32, tag="ps")
                for hk in range(HKT):
                    nc.tensor.matmul(
                        ps[:],
                        lhsT=xt_chunk(tau, hk)[:, tb_local * KBLK:(tb_local + 1) * KBLK],
                        rhs=w_chunk("wv", hk),
                        start=(hk == 0), stop=(hk == HKT - 1))
                nc.vector.tensor_copy(
                    out=v_sb[:, tau * (TBLK // KBLK) + tb_local, :], in_=ps[:])

            mask_tiles = {}

            def emit_mask_loads(tau):
                tsl = slice(tau * TBLK, (tau + 1) * TBLK)
                for run in all_runs[tau]:
                    mt = mask_pool.tile([128, max_run_len, TBLK], bf16,
                                        tag="mask")
                    nc.sync.dma_start(
                        mt[:, :len(run), :],
                        maskT.ap()[run[0] * KBLK:(run[-1] + 1) * KBLK, tsl]
                        .rearrange("(k p) t -> p k t", p=128))
                    for j, Tb in enumerate(run):
                        mask_tiles[(tau, Tb)] = mt[:, j, :]

            def emit_attention_head(tau, h):
                tsl = slice(tau * TBLK, (tau + 1) * TBLK)
                blocks = pattern[tau]
                od = ps_acc.tile([128, TBLK], fp32, tag="od")
                esum = esum_pool.tile([128, TBLK], fp32, tag="esum")
                for i, (Tb, partial) in enumerate(blocks):
                    sp = ps_work.tile([128, TBLK], fp32, tag="ps")
                    nc.tensor.matmul(
                        sp[:],
                        lhsT=kT_sb[:, h, Tb * KBLK:(Tb + 1) * KBLK],
                        rhs=qT_sb[:, h, tsl],
                        start=True, stop=True)
                    e = e_pool.tile([128, TBLK], bf16, tag="e")
                    nc.scalar.activation(out=e[:], in_=sp[:], func=Exp,
                                         scale=inv_sqrt_hd)
                    if partial:
                        nc.vector.tensor_mul(e[:], e[:], mask_tiles[(tau, Tb)])
                    if i == 0:
                        nc.vector.tensor_copy(out=esum[:], in_=e[:])
                    else:
                        nc.vector.tensor_add(esum[:], esum[:], e[:])
                    nc.tensor.matmul(
                        od[:],
                        lhsT=v_sb[:, Tb, h * HD:(h + 1) * HD],
                        rhs=e[:],
                        start=(i == 0), stop=(i == len(blocks) - 1))
                dn = ps_den.tile([1, TBLK], fp32, tag="dn")
                nc.tensor.matmul(dn[:], lhsT=ones32_sb[:], rhs=esum[:],
                                 start=True, stop=True)
                r = r_pool.tile([1, TBLK], fp32, tag="r")
                nc.vector.reciprocal_approx_fast(out=r[:], in_=dn[:])
                R = R_pool.tile([128, TBLK], fp32, tag="R")
                nc.gpsimd.partition_broadcast(R[:], r[:])
                nc.vector.tensor_mul(oT_sb[:, h, tsl], od[:], R[:])

            def emit_out_row(tt):
                # one 128-row slab of the final projection + store
                for half in range(2):
                    osb = out_pool.tile([128, H // 2], fp32, tag="osb")
                    for mbl in range(H // TBLK // 2):
                        mb = half * (H // TBLK // 2) + mbl
                        ps = ps_work.tile([128, TBLK], fp32, tag="ps")
                        for h in range(HPC):
                            nc.tensor.matmul(
                                ps[:],
                                lhsT=oT_sb[:, h, tt * 128:(tt + 1) * 128],
                                rhs=wo_sb[:, h, mb * TBLK:(mb + 1) * TBLK],
                                start=(h == 0), stop=(h == HPC - 1))
                        nc.vector.tensor_copy(
                            out=osb[:, mbl * TBLK:(mbl + 1) * TBLK], in_=ps[:])
                    nc.gpsimd.dma_start(
                        out.ap()[tt * 128:(tt + 1) * 128,
                                 half * (H // 2):(half + 1) * (H // 2)],
                        osb[:])

            # ---- emission schedule -----------------------------------
            # phase3 rows of block tau are deferred: half into slot
            # tau+1, half into slot tau+2 (clamped), so the PE has filler
            # work inside the ACT-bound attention stretches, including
            # the long final one.
            rows_per_tau = TBLK // 128
            p3_assign = {t: [] for t in range(NT + 1)}  # slot -> tt list
            for ptau in range(NT):
                rows = list(range(ptau * rows_per_tau, (ptau + 1) * rows_per_tau))
                s1 = min(ptau + 1, NT)
                s2 = min(ptau + 2, NT)
                p3_assign[s1].extend(rows[:rows_per_tau // 2])
                p3_assign[s2].extend(rows[rows_per_tau // 2:])

            emit_xt_load(0)
            emit_mask_loads(0)
            # projections for tau=0 run standalone (startup)
            for h in range(HPC):
                emit_qk_proj(0, "wq", h)
            for h in range(HPC):
                emit_qk_proj(0, "wk", h)
            for tb in range(rows_per_tau):
                emit_v_proj(0, tb)

            for tau in range(NT):
                fillers = []
                if tau + 1 < NT:
                    emit_xt_load(tau + 1)
                    emit_mask_loads(tau + 1)
                    fillers += [lambda h=h, t=tau + 1: emit_qk_proj(t, "wq", h)
                                for h in range(HPC)]
                    fillers += [lambda h=h, t=tau + 1: emit_qk_proj(t, "wk", h)
                                for h in range(HPC)]
                    fillers += [lambda tb=tb, t=tau + 1: emit_v_proj(t, tb)
                                for tb in range(rows_per_tau)]
                fillers += [lambda tt=tt: emit_out_row(tt)
                            for tt in p3_assign[tau]]
                primary = [lambda h=h, t=tau: emit_attention_head(t, h)
                           for h in range(HPC)]
                for unit in _interleave(primary, fillers):
                    unit()
                if tau == 0:
                    # wo rides the scalar queue behind tau=0's exps --
                    # out of the startup bandwidth window, but done long
                    # before the first deferred phase-3 row needs it.
                    nc.scalar.dma_start(
                        wo_sb[:],
                        wo.ap().rearrange("(c p) m -> p c m", p=128))

            for tt in p3_assign[NT]:
                emit_out_row(tt)

    nc.compile()
    return nc


def _classify(mask):
    """Per 128x512 block of mask^T: skip / full / partial, unioned over
    batches.  Returns the pattern tuple, or None if some row is fully
    masked (degenerate -- reference gives uniform weights there)."""
    if not mask.any(axis=2).all():
        return None
    pattern = []
    for tau in range(NT):
        blocks = []
        for Tb in range(NK):
            # block of mask^T[Tb*128:(Tb+1)*128, tau*512:(tau+1)*512]
            # == mask[:, tau*512:(tau+1)*512, Tb*128:(Tb+1)*128]
            blk = mask[:, tau * TBLK:(tau + 1) * TBLK,
                       Tb * KBLK:(Tb + 1) * KBLK]
            if not blk.any():
                continue
            blocks.append((Tb, not blk.all()))
        pattern.append(tuple(blocks))
    return tuple(pattern)


def _reference_fallback(x, mask, Wq, Wk, Wv, Wo):
    out = np.empty((B, S, H), np.float32)
    for b in range(B):
        q = (x[b] @ Wq).reshape(S, NH, HD).transpose(1, 0, 2)
        k = (x[b] @ Wk).reshape(S, NH, HD).transpose(1, 0, 2)
        v = (x[b] @ Wv).reshape(S, NH, HD).transpose(1, 0, 2)
        s = np.einsum("htd,hTd->htT", q, k) / np.sqrt(HD)
        s = np.where(mask[b][None], s, -1e10)
        s -= s.max(-1, keepdims=True)
        w = np.exp(s)
        w /= w.sum(-1, keepdims=True)
        o = np.einsum("htT,hTd->htd", w, v)
        out[b] = o.transpose(1, 0, 2).reshape(S, NH * HD) @ Wo
    return out


def kernel(x, mask, Wq, Wk, Wv, Wo):
    x = np.asarray(x, np.float32)
    mask = np.asarray(mask).astype(bool)
    Wq = np.asarray(Wq, np.float32)
    Wk = np.asarray(Wk, np.float32)
    Wv = np.asarray(Wv, np.float32)
    Wo = np.asarray(Wo, np.float32)
    assert x.shape == (B, S, H) and mask.shape == (B, S, S)

    pattern = _classify(mask)
    if pattern is None:
        return _reference_fallback(x, mask, Wq, Wk, Wv, Wo)

    if pattern not in _kernel_cache:
        _kernel_cache[pattern] = _build(pattern)
    nc = _kernel_cache[pattern]

    xT_b = [np.ascontiguousarray(x[b].T).astype(_BF16) for b in range(B)]
    maskT_b = [np.ascontiguousarray(mask[b].T).astype(_BF16) for b in range(B)]
    wq_g = [np.ascontiguousarray(Wq[:, g * DPC:(g + 1) * DPC]).astype(_BF16)
            for g in range(GROUPS)]
    wk_g = [np.ascontiguousarray(Wk[:, g * DPC:(g + 1) * DPC]).astype(_BF16)
            for g in range(GROUPS)]
    wv_g = [np.ascontiguousarray(Wv[:, g * DPC:(g + 1) * DPC]).astype(_BF16)
            for g in range(GROUPS)]
    wo_g = [np.ascontiguousarray(Wo[g * DPC:(g + 1) * DPC, :]).astype(_BF16)
            for g in range(GROUPS)]

    in_maps = []
    for i in range(N_CORES):
        b, g = divmod(i, GROUPS)
        in_maps.append({
            "xT": xT_b[b], "maskT": maskT_b[b],
            "wq": wq_g[g], "wk": wk_g[g], "wv": wv_g[g], "wo": wo_g[g],
        })

    from concourse.bass_utils import run_bass_kernel_spmd
    res = run_bass_kernel_spmd(nc, in_maps, core_ids=list(range(N_CORES)))

    out = np.zeros((B, S, H), np.float32)
    for i in range(N_CORES):
        b = i // GROUPS
        out[b] += res.results[i]["out"]
    return out


# revision 12
# speedup vs baseline: 1.2823x; 1.0212x over previous
"""Multi-head attention (B=2, S=2048, H=2048, NH=16, HD=128) on 8 trn2 cores.

Sharding: core i -> (batch b = i // 4, head-group g = i % 4, 4 heads each).
Each core computes q/k/v projections for its 4 heads, causal-masked
attention, and a partial output projection against its 512-row slice of
Wo.  The host sums the 4 partial outputs per batch.

Layout strategy (everything K-major so no on-chip transposes are needed):
  - host ships x^T (per batch) in bf16; projections compute q^T/k^T
    [d, t] via lhsT=W, rhs=x^T, and v [T, d] via lhsT=x^T, rhs=Wv.
  - scores^T [T, t] = (k^T).T @ q^T; exp on ACT (no max subtraction --
    scores are O(6) here, exp is safe in fp32); runtime mask applied
    multiplicatively AFTER exp (so softmax denominators stay exact).
  - softmax denominators: e tiles accumulate on DVE into an fp32 esum,
    reduced across partitions with one ones-matmul per (head, block);
    o^T [d, t] = v.T @ e accumulates in PSUM; normalized by broadcast
    reciprocal on the way out to SBUF.
  - final: out[t, m] = (o^T).T @ Wo_rows, accumulated over the 4 heads.

The mask is inspected on the host and the kernel is specialized per
128x512 block: skip (all False), full (all True), or partial (loads the
mask tile and multiplies).  For the causal mask this halves attention
FLOPs; for an all-ones mask it becomes a dense kernel automatically.

Emission is software-pipelined: in query-block tau's slot we emit its
attention heads round-robin with the projections of tau+1 and deferred
output-projection rows, so the PE always has independent matmul work
while ACT grinds through the exps.
"""

import math

import numpy as np
import ml_dtypes

B, S, H, NH, HD = 2, 2048, 2048, 16, 128
N_CORES = 8
GROUPS = 4                # head-groups (cores per batch)
HPC = NH // GROUPS        # heads per core = 4
DPC = HPC * HD            # head dims per core = 512
TBLK = 512                # query-block width (matmul moving dim)
KBLK = 128                # key-block width (matmul contraction dim)
NT = S // TBLK            # 4 query blocks
NK = S // KBLK            # 16 key blocks
HKT = H // 128            # 16 contraction tiles over hidden dim
HKC = 4                   # contraction chunks per DMA (so loads pipeline)

_BF16 = ml_dtypes.bfloat16

_kernel_cache = {}


def _runs(blocks):
    """Group the partial blocks of one query block into contiguous Tb
    runs so each run loads with a single DMA."""
    runs = []
    for Tb, partial in blocks:
        if not partial:
            continue
        if runs and runs[-1][-1] == Tb - 1:
            runs[-1].append(Tb)
        else:
            runs.append([Tb])
    return runs


def _interleave(primary, fillers):
    """Round-robin: after primary unit i, its even share of fillers."""
    out = []
    n = max(len(primary), 1)
    for i, p in enumerate(primary):
        out.append(p)
        out.extend(fillers[i * len(fillers) // n:(i + 1) * len(fillers) // n])
    out.extend(fillers[len(primary) * len(fillers) // n:])
    return out


def _build(pattern):
    """Compile the SPMD program for a given mask block pattern.

    pattern: tuple over query-block tau of tuples of (Tb, partial) pairs,
    ascending in Tb, listing key blocks that have any visible entry.
    """
    import concourse.bass as bass  # noqa: F401
    import concourse.tile as tile
    from concourse import bacc, mybir

    fp32 = mybir.dt.float32
    bf16 = mybir.dt.bfloat16
    Exp = mybir.ActivationFunctionType.Exp
    inv_sqrt_hd = 1.0 / math.sqrt(HD)

    all_runs = [_runs(blocks) for blocks in pattern]
    max_run_len = max((len(r) for runs in all_runs for r in runs), default=1)
    max_runs = max((len(runs) for runs in all_runs), default=1)

    nc = bacc.Bacc("TRN2", target_bir_lowering=False, debug=False,
                   num_devices=N_CORES)
    xT = nc.dram_tensor("xT", [H, S], bf16, kind="ExternalInput")
    wq = nc.dram_tensor("wq", [H, DPC], bf16, kind="ExternalInput")
    wk = nc.dram_tensor("wk", [H, DPC], bf16, kind="ExternalInput")
    wv = nc.dram_tensor("wv", [H, DPC], bf16, kind="ExternalInput")
    wo = nc.dram_tensor("wo", [DPC, H], bf16, kind="ExternalInput")
    maskT = nc.dram_tensor("maskT", [S, S], bf16, kind="ExternalInput")
    out = nc.dram_tensor("out", [S, H], fp32, kind="ExternalOutput")

    n_chunks = HKT // HKC  # 4

    with tile.TileContext(nc) as tc:
        with (
            tc.tile_pool(name="persist", bufs=1) as persist,
            tc.tile_pool(name="xt", bufs=6) as xt_pool,
            tc.tile_pool(name="masks", bufs=max(2 * max_runs, 2)) as mask_pool,
            tc.tile_pool(name="e", bufs=10) as e_pool,
            tc.tile_pool(name="outsb", bufs=2) as out_pool,
            tc.tile_pool(name="esum", bufs=3) as esum_pool,
            tc.tile_pool(name="rp", bufs=2) as r_pool,
            tc.tile_pool(name="Rp", bufs=2) as R_pool,
            tc.tile_pool(name="ps_work", bufs=4, space="PSUM") as ps_work,
            tc.tile_pool(name="ps_acc", bufs=3, space="PSUM") as ps_acc,
            tc.tile_pool(name="ps_den", bufs=1, space="PSUM") as ps_den,
        ):
            # --- persistent SBUF tensors -------------------------------
            # DMA queue discipline: sync carries the latency-critical
            # steady loads (xT blocks, masks), gpsimd the weights at
            # startup plus output stores, scalar only wo (emitted late --
            # it queues behind the first exps, landing well before
            # phase 3 needs it).  Never tensor: its sequencer must stay
            # dedicated to the matmul stream.
            WCH = 2  # contraction tiles per weight-load chunk
            w_sbs = {}
            for name, dram, eng in (("wq", wq, nc.gpsimd), ("wk", wk, nc.sync),
                                    ("wv", wv, nc.scalar)):
                chunks = []
                for c in range(HKT // WCH):
                    t = persist.tile([128, WCH, DPC], bf16, tag=f"{name}{c}")
                    eng.dma_start(
                        t[:],
                        dram.ap()[c * WCH * 128:(c + 1) * WCH * 128, :]
                        .rearrange("(k p) d -> p k d", p=128))
                    chunks.append(t)
                w_sbs[name] = chunks
            wo_sb = persist.tile([128, HPC, H], bf16, tag="wo")

            qT_sb = persist.tile([128, HPC, S], bf16, tag="qT")
            kT_sb = persist.tile([128, HPC, S], bf16, tag="kT")
            v_sb = persist.tile([128, NK, DPC], bf16, tag="v")
            oT_sb = persist.tile([128, HPC, S], bf16, tag="oT")

            f32r = mybir.dt.float32r
            ones_f32_sb = persist.tile([128, 1], fp32, tag="ones_f32")
            nc.vector.memset(ones_f32_sb[:], 1.0)
            ones32_sb = persist.tile([128, 1], f32r, tag="ones32")
            nc.vector.tensor_copy(out=ones32_sb[:], in_=ones_f32_sb[:])

            def w_chunk(name, hk):
                return w_sbs[name][hk // WCH][:, hk % WCH, :]

            xts = {}

            def emit_xt_load(tau):
                tsl = slice(tau * TBLK, (tau + 1) * TBLK)
                xts[tau] = []
                for c in range(n_chunks):
                    t = xt_pool.tile([128, HKC, TBLK], bf16, tag="xt")
                    nc.sync.dma_start(
                        t[:],
                        xT.ap()[c * HKC * 128:(c + 1) * HKC * 128, tsl]
                        .rearrange("(k p) t -> p k t", p=128))
                    xts[tau].append(t)

            def xt_chunk(tau, hk):
                return xts[tau][hk // HKC][:, hk % HKC, :]

            def emit_qk_proj(tau, wname, h):
                tsl = slice(tau * TBLK, (tau + 1) * TBLK)
                dst = qT_sb if wname == "wq" else kT_sb
                ps = ps_work.tile([128, TBLK], fp32, tag="ps")
                for hk in range(HKT):
                    nc.tensor.matmul(
                        ps[:],
                        lhsT=w_chunk(wname, hk)[:, h * HD:(h + 1) * HD],
                        rhs=xt_chunk(tau, hk),
                        start=(hk == 0), stop=(hk == HKT - 1))
                nc.vector.tensor_copy(out=dst[:, h, tsl], in_=ps[:])

            def emit_v_proj(tau, tb_local):
                ps = ps_work.tile([128, TBLK], fp32, tag="ps")
                for hk in range(HKT):
                    nc.tensor.matmul(
                        ps[:],
                        lhsT=xt_chunk(tau, hk)[:, tb_local * KBLK:(tb_local + 1) * KBLK],
                        rhs=w_chunk("wv", hk),
                        start=(hk == 0), stop=(hk == HKT - 1))
                nc.vector.tensor_copy(
                    out=v_sb[:, tau * (TBLK // KBLK) + tb_local, :], in_=ps[:])

            mask_tiles = {}

            def emit_mask_loads(tau):
                tsl = slice(tau * TBLK, (tau + 1) * TBLK)
                for run in all_runs[tau]:
                    mt = mask_pool.tile([128, max_run_len, TBLK], bf16,
                                        tag="mask")
                    nc.sync.dma_start(
                        mt[:, :len(run), :],
                        maskT.ap()[run[0] * KBLK:(run[-1] + 1) * KBLK, tsl]
                        .rearrange("(k p) t -> p k t", p=128))
                    for j, Tb in enumerate(run):
                        mask_tiles[(tau, Tb)] = mt[:, j, :]

            def emit_attention_head(tau, h):
                tsl = slice(tau * TBLK, (tau + 1) * TBLK)
                blocks = pattern[tau]
                od = ps_acc.tile([128, TBLK], fp32, tag="od")
                esum = esum_pool.tile([128, TBLK], f32r, tag="esum")
                for i, (Tb, partial) in enumerate(blocks):
                    sp = ps_work.tile([128, TBLK], fp32, tag="ps")
                    nc.tensor.matmul(
                        sp[:],
                        lhsT=kT_sb[:, h, Tb * KBLK:(Tb + 1) * KBLK],
                        rhs=qT_sb[:, h, tsl],
                        start=True, stop=True)
                    e = e_pool.tile([128, TBLK], bf16, tag="e")
                    nc.scalar.activation(out=e[:], in_=sp[:], func=Exp,
                                         scale=inv_sqrt_hd)
                    if partial:
                        nc.vector.tensor_mul(e[:], e[:], mask_tiles[(tau, Tb)])
                    if i == 0:
                        nc.vector.tensor_copy(out=esum[:], in_=e[:])
                    else:
                        nc.vector.tensor_add(esum[:], esum[:], e[:])
                    nc.tensor.matmul(
                        od[:],
                        lhsT=v_sb[:, Tb, h * HD:(h + 1) * HD],
                        rhs=e[:],
                        start=(i == 0), stop=(i == len(blocks) - 1))
                dn = ps_den.tile([1, TBLK], fp32, tag="dn")
                nc.tensor.matmul(dn[:], lhsT=ones32_sb[:], rhs=esum[:],
                                 start=True, stop=True)
                r = r_pool.tile([1, TBLK], fp32, tag="r")
                nc.vector.reciprocal_approx_fast(out=r[:], in_=dn[:])
                R = R_pool.tile([128, TBLK], fp32, tag="R")
                nc.gpsimd.partition_broadcast(R[:], r[:])
                nc.vector.tensor_mul(oT_sb[:, h, tsl], od[:], R[:])

            def emit_out_row(tt):
                # one 128-row slab of the final projection + store
                for half in range(2):
                    osb = out_pool.tile([128, H // 2], fp32, tag="osb")
                    for mbl in range(H // TBLK // 2):
                        mb = half * (H // TBLK // 2) + mbl
                        ps = ps_work.tile([128, TBLK], fp32, tag="ps")
                        for h in range(HPC):
                            nc.tensor.matmul(
                                ps[:],
                                lhsT=oT_sb[:, h, tt * 128:(tt + 1) * 128],
                                rhs=wo_sb[:, h, mb * TBLK:(mb + 1) * TBLK],
                                start=(h == 0), stop=(h == HPC - 1))
                        nc.vector.tensor_copy(
                            out=osb[:, mbl * TBLK:(mbl + 1) * TBLK], in_=ps[:])
                    nc.gpsimd.dma_start(
                        out.ap()[tt * 128:(tt + 1) * 128,
                                 half * (H // 2):(half + 1) * (H // 2)],
                        osb[:])

            # ---- emission schedule -----------------------------------
            # phase3 rows of block tau are deferred: half into slot
            # tau+1, half into slot tau+2 (clamped), so the PE has filler
            # work inside the ACT-bound attention stretches, including
            # the long final one.
            rows_per_tau = TBLK // 128
            p3_assign = {t: [] for t in range(NT + 1)}  # slot -> tt list
            for ptau in range(NT):
                rows = list(range(ptau * rows_per_tau, (ptau + 1) * rows_per_tau))
                s1 = min(ptau + 1, NT)
                s2 = min(ptau + 2, NT)
                p3_assign[s1].extend(rows[:rows_per_tau // 2])
                p3_assign[s2].extend(rows[rows_per_tau // 2:])

            emit_xt_load(0)
            emit_mask_loads(0)
            # projections for tau=0 run standalone (startup)
            for h in range(HPC):
                emit_qk_proj(0, "wq", h)
            for h in range(HPC):
                emit_qk_proj(0, "wk", h)
            for tb in range(rows_per_tau):
                emit_v_proj(0, tb)

            for tau in range(NT):
                fillers = []
                if tau + 1 < NT:
                    emit_xt_load(tau + 1)
                    emit_mask_loads(tau + 1)
                    fillers += [lambda h=h, t=tau + 1: emit_qk_proj(t, "wq", h)
                                for h in range(HPC)]
                    fillers += [lambda h=h, t=tau + 1: emit_qk_proj(t, "wk", h)
                                for h in range(HPC)]
                    fillers += [lambda tb=tb, t=tau + 1: emit_v_proj(t, tb)
                                for tb in range(rows_per_tau)]
                fillers += [lambda tt=tt: emit_out_row(tt)
                            for tt in p3_assign[tau]]
                primary = [lambda h=h, t=tau: emit_attention_head(t, h)
                           for h in range(HPC)]
                for unit in _interleave(primary, fillers):
                    unit()
                if tau == 0:
                    # wo rides the scalar queue behind tau=0's exps --
                    # out of the startup bandwidth window, but done long
                    # before the first deferred phase-3 row needs it.
                    nc.scalar.dma_start(
                        wo_sb[:],
                        wo.ap().rearrange("(c p) m -> p c m", p=128))

            for tt in p3_assign[NT]:
                emit_out_row(tt)

    nc.compile()
    return nc


def _classify(mask):
    """Per 128x512 block of mask^T: skip / full / partial, unioned over
    batches.  Returns the pattern tuple, or None if some row is fully
    masked (degenerate -- reference gives uniform weights there)."""
    if not mask.any(axis=2).all():
        return None
    pattern = []
    for tau in range(NT):
        blocks = []
        for Tb in range(NK):
            # block of mask^T[Tb*128:(Tb+1)*128, tau*512:(tau+1)*512]
            # == mask[:, tau*512:(tau+1)*512, Tb*128:(Tb+1)*128]
            blk = mask[:, tau * TBLK:(tau + 1) * TBLK,
                       Tb * KBLK:(Tb + 1) * KBLK]
            if not blk.any():
                continue
            blocks.append((Tb, not blk.all()))
        pattern.append(tuple(blocks))
    return tuple(pattern)


def _reference_fallback(x, mask, Wq, Wk, Wv, Wo):
    out = np.empty((B, S, H), np.float32)
    for b in range(B):
        q = (x[b] @ Wq).reshape(S, NH, HD).transpose(1, 0, 2)
        k = (x[b] @ Wk).reshape(S, NH, HD).transpose(1, 0, 2)
        v = (x[b] @ Wv).reshape(S, NH, HD).transpose(1, 0, 2)
        s = np.einsum("htd,hTd->htT", q, k) / np.sqrt(HD)
        s = np.where(mask[b][None], s, -1e10)
        s -= s.max(-1, keepdims=True)
        w = np.exp(s)
        w /= w.sum(-1, keepdims=True)
        o = np.einsum("htT,hTd->htd", w, v)
        out[b] = o.transpose(1, 0, 2).reshape(S, NH * HD) @ Wo
    return out


def kernel(x, mask, Wq, Wk, Wv, Wo):
    x = np.asarray(x, np.float32)
    mask = np.asarray(mask).astype(bool)
    Wq = np.asarray(Wq, np.float32)
    Wk = np.asarray(Wk, np.float32)
    Wv = np.asarray(Wv, np.float32)
    Wo = np.asarray(Wo, np.float32)
    assert x.shape == (B, S, H) and mask.shape == (B, S, S)

    pattern = _classify(mask)
    if pattern is None:
        return _reference_fallback(x, mask, Wq, Wk, Wv, Wo)

    if pattern not in _kernel_cache:
        _kernel_cache[pattern] = _build(pattern)
    nc = _kernel_cache[pattern]

    xT_b = [np.ascontiguousarray(x[b].T).astype(_BF16) for b in range(B)]
    maskT_b = [np.ascontiguousarray(mask[b].T).astype(_BF16) for b in range(B)]
    wq_g = [np.ascontiguousarray(Wq[:, g * DPC:(g + 1) * DPC]).astype(_BF16)
            for g in range(GROUPS)]
    wk_g = [np.ascontiguousarray(Wk[:, g * DPC:(g + 1) * DPC]).astype(_BF16)
            for g in range(GROUPS)]
    wv_g = [np.ascontiguousarray(Wv[:, g * DPC:(g + 1) * DPC]).astype(_BF16)
            for g in range(GROUPS)]
    wo_g = [np.ascontiguousarray(Wo[g * DPC:(g + 1) * DPC, :]).astype(_BF16)
            for g in range(GROUPS)]

    in_maps = []
    for i in range(N_CORES):
        b, g = divmod(i, GROUPS)
        in_maps.append({
            "xT": xT_b[b], "maskT": maskT_b[b],
            "wq": wq_g[g], "wk": wk_g[g], "wv": wv_g[g], "wo": wo_g[g],
        })

    from concourse.bass_utils import run_bass_kernel_spmd
    res = run_bass_kernel_spmd(nc, in_maps, core_ids=list(range(N_CORES)))

    out = np.zeros((B, S, H), np.float32)
    for i in range(N_CORES):
        b = i // GROUPS
        out[b] += res.results[i]["out"]
    return out
